# revision 1
# baseline (speedup 1.0000x reference)
"""BERT layer (B=8, S=512, H=768, NH=12, DH=64, FF=3072) on 8 Trainium2 cores.

Data-parallel over batch (1 element/core).  Feature-major on-chip layout
(activations as X^T [H partitions, S free]).  All contraction>=256 matmuls run
fp8e4m3 DoubleRow (2x PE rate): QKV projections, ctx/den, Wo, FFN1, FFN2.
Weights are host-scaled x64 so fp8 stays in normal range; the 1/64 unscale is
folded into the PSUM-evacuation ops that exist anyway.  Scores stay bf16
(K=64, row-packed head pairs); LayerNorm stats stay f32r.

Softmax: the additive 0/1 mask is folded multiplicatively (exp(s+mb)=exp(s)*m):
V is scaled by m/64... (mask/64 on V evac), and the denominator matmul's
stationary operand is mask/64 (2^-6, exact in fp8), so exp needs no bias and
runs as 24 batched [128,1024] ACT ops straight out of 2-bank PSUM tiles.
recip = 64/den cancels the 1/64 and lands ctx at x64 scale, which is exactly
the fp8-friendly range for the Wo input.

LayerNorm rstd = exp(-0.5*ln(var+eps)) keeps the ACT table set at
natural_log_exp_and_others (shared with softmax exp): only 2 table switches
per layer (to/from the Gelu set) instead of 4.

Bias folding (all exact):
  bq,bk: added at Q/K PSUM evacuation (tensor_scalar).
  bv,bo: bo_eff = bo + bv@Wo becomes LN1's per-partition shift, applied via
         the Square bias + the cen subtraction + a mean offset (zero ops).
  ln1_b (beta1): h1 = gamma1*nrm + beta1. The fp8 h1 fed to FFN1 omits beta1
         (compensated by b1_eff = b1 + beta1@W1); the residual carries
         q = gamma1*nrm exactly, with beta1 folded into LN2's shift
         c2 = b2 + beta1.
"""

from contextlib import ExitStack

import numpy as np
import ml_dtypes

from concourse import bacc
import concourse.tile as tile
from concourse import mybir
from concourse.bass_utils import run_bass_kernel_spmd

F32 = mybir.dt.float32
F32R = mybir.dt.float32r
BF16 = mybir.dt.bfloat16
F8 = mybir.dt.float8e4
AF = mybir.ActivationFunctionType
ALU = mybir.AluOpType
PM = mybir.MatmulPerfMode

B, S, H, NH, DH, FF = 8, 512, 768, 12, 64, 3072
EPS = 1e-3
CH = H // 128   # 6 hidden chunks
CF = FF // 128  # 24 ff chunks
T = S // 128    # 4 token/key chunks
NP = NH // 2    # 6 head pairs
FFN_MODE = "bf16"  # "hilo" | "mix" | "bf16"
SW = 64.0       # weight scale (keeps fp8 in normal range)
ISW = 1.0 / SW

# consts tile column map: [128, NCONST]
BQ, BK, BO, L1G, C2, L2G, L2B = 0, 6, 12, 18, 24, 30, 36
MCOL = 42        # 4 cols: mask/64 per key chunk (for the V evacuation)
B1E = 46         # 24 cols: b1 + ln1_b @ W1
NCONST = B1E + CF
# scal tile [128, 2]: col 0 = sum(bo_eff)/H, col 1 = sum(b2+ln1_b)/H


def ts(i, n):
    return slice(i * n, (i + 1) * n)


def build_nc(repeats=1, rstd_mode="lnexp", ffn_mode=None):
    ffn_mode = ffn_mode or FFN_MODE
    nc = bacc.Bacc("TRN2", target_bir_lowering=False, debug=False)

    xT_d = nc.declare_dram_parameter("xT", [H, S], F32R, isOutput=False)
    xT8_d = nc.declare_dram_parameter("xT8", [H, S], F8, isOutput=False)
    wqk_d = nc.declare_dram_parameter("wqkb", [CH, 128, 2, CH, 128], F8,
                                      isOutput=False)
    wv_d = nc.declare_dram_parameter("wv", [H, H], F8, isOutput=False)
    wob_d = nc.declare_dram_parameter("wob", [CH, 128, CH, 128], F8,
                                      isOutput=False)
    # FFN weights: "hilo" = hi+lo fp8 residual planes, "bf16" = plain bf16,
    # "mix" = ff1 single fp8 + ff2 bf16
    W1DT = F8 if ffn_mode in ("hilo", "mix") else BF16
    W2DT = F8 if ffn_mode == "hilo" else BF16
    W1P = 2 * CH if ffn_mode == "hilo" else CH
    W2P = 2 * CF if ffn_mode == "hilo" else CF
    w1_d = nc.declare_dram_parameter("w1b", [CF, 128, W1P, 128], W1DT,
                                     isOutput=False)
    w2_d = nc.declare_dram_parameter("w2b", [CH, 128, W2P, 128], W2DT,
                                     isOutput=False)
    mden_d = nc.declare_dram_parameter("mden", [128, T, 64], F8,
                                       isOutput=False)
    c_d = nc.declare_dram_parameter("consts", [128, NCONST], F32,
                                    isOutput=False)
    sc_d = nc.declare_dram_parameter("scal", [128, 2], F32, isOutput=False)
    out_d = nc.declare_dram_parameter("outT", [H, S], F32, isOutput=True)

    def fmaj(d):
        return d.rearrange("(i p) n -> p i n", p=128)

    with tile.TileContext(nc) as tc, ExitStack() as top:
        cpool = top.enter_context(tc.tile_pool(name="cpool", bufs=1))
        c_sb = cpool.tile([128, NCONST], F32, name="c_sb")
        nc.sync.dma_start(out=c_sb, in_=c_d[:, :])
        sc_sb = cpool.tile([128, 2], F32, name="sc_sb")
        nc.sync.dma_start(out=sc_sb, in_=sc_d[:, :])
        mden = cpool.tile([128, T, 64], F8, name="mden")
        nc.sync.dma_start(out=mden, in_=mden_d[:, :, :])
        ones_f32 = cpool.tile([128, 128], F32, name="ones_f32")
        nc.vector.memset(ones_f32, 1.0)
        ones_sum = cpool.tile([128, 128], F32R, name="ones_sum")
        nc.vector.tensor_copy(out=ones_sum, in_=ones_f32)

        mid = top.enter_context(tc.tile_pool(name="mid", bufs=1))
        tmp = top.enter_context(tc.tile_pool(name="tmp", bufs=1))
        fpool = top.enter_context(tc.tile_pool(name="fpool", bufs=1))
        w1pool = top.enter_context(tc.tile_pool(name="w1p", bufs=4))
        w2pool = top.enter_context(tc.tile_pool(name="w2p", bufs=2))

        def layer_norm(pssum, shift_col, msh_col, src, emit, cen_pool=False):
            """LN over features of (src + shift); emit(jj, cen_ap, rstd)."""
            sum_ps = pssum.tile([128, S], F32, tag="lnsum", bufs=1,
                                name="sum_ps")
            sq_ps = pssum.tile([128, S], F32, tag="lnsq", bufs=1, name="sq_ps")
            for i in range(CH):
                nc.tensor.matmul(sum_ps[:, :], ones_sum[:, :], src[:, i, :],
                                 start=(i == 0), stop=(i == CH - 1))
            for i in range(CH):
                sq = tmp.tile([128, S], F32R, tag="sq", bufs=3, name="sq")
                nc.scalar.activation(
                    out=sq, in_=src[:, i, :], func=AF.Square,
                    bias=c_sb[:, shift_col + i:shift_col + i + 1])
                nc.tensor.matmul(sq_ps[:, :], ones_sum[:, :], sq,
                                 start=(i == 0), stop=(i == CH - 1))
            mean = tmp.tile([128, S], F32, tag="mean", bufs=1, name="mean")
            nc.vector.tensor_scalar(
                out=mean, in0=sum_ps[:, :], scalar1=1.0 / H,
                scalar2=sc_sb[:, msh_col:msh_col + 1],
                op0=ALU.mult, op1=ALU.add)
            m2 = tmp.tile([128, S], F32, tag="m2", bufs=1, name="m2")
            nc.scalar.activation(out=m2, in_=mean, func=AF.Square)
            # var+eps: (sq_ps/H + eps) - mean^2
            var = tmp.tile([128, S], F32, tag="var", bufs=1, name="var")
            ve = tmp.tile([128, S], F32, tag="ve", bufs=1, name="ve")
            nc.vector.tensor_scalar(
                out=ve, in0=sq_ps[:, :], scalar1=1.0 / H, scalar2=EPS,
                op0=ALU.mult, op1=ALU.add)
            nc.vector.tensor_tensor(out=var, in0=ve, in1=m2, op=ALU.subtract)
            rstd = tmp.tile([128, S], F32, tag="rstd", bufs=1, name="rstd")
            if rstd_mode == "lnexp":
                lnv = tmp.tile([128, S], F32, tag="lnv", bufs=1, name="lnv")
                nc.scalar.activation(out=lnv, in_=var, func=AF.Ln)
                nc.scalar.activation(out=rstd, in_=lnv, func=AF.Exp,
                                     scale=-0.5)
            else:
                # quake rsqrt: bit-trick seed + 2 Newton iterations
                iv = var.bitcast(mybir.dt.int32)
                sh = tmp.tile([128, S], mybir.dt.int32, tag="qsh", bufs=1,
                              name="qsh")
                nc.vector.tensor_scalar(
                    out=sh, in0=iv, scalar1=1, scalar2=None,
                    op0=ALU.logical_shift_right)
                sh2 = tmp.tile([128, S], mybir.dt.int32, tag="qsh2", bufs=1,
                               name="qsh2")
                nc.vector.tensor_scalar(
                    out=sh2, in0=sh, scalar1=0x5f3759df + 1, scalar2=None,
                    op0=ALU.subtract)
                y0i = tmp.tile([128, S], mybir.dt.int32, tag="qy0", bufs=1,
                               name="qy0")
                nc.vector.tensor_scalar(
                    out=y0i, in0=sh2, scalar1=-1, scalar2=None,
                    op0=ALU.bitwise_xor)
                y = y0i.bitcast(F32)
                for it in range(2):
                    a = tmp.tile([128, S], F32, tag="qa", bufs=1,
                                 name="qa")
                    nc.vector.tensor_tensor(out=a, in0=y, in1=y, op=ALU.mult)
                    bq_ = tmp.tile([128, S], F32, tag="qb", bufs=1,
                                   name="qb")
                    nc.vector.tensor_tensor(out=bq_, in0=a, in1=var,
                                            op=ALU.mult)
                    cq = tmp.tile([128, S], F32, tag="qc", bufs=1,
                                  name="qc")
                    nc.vector.tensor_scalar(
                        out=cq, in0=bq_, scalar1=-0.5, scalar2=1.5,
                        op0=ALU.mult, op1=ALU.add)
                    dst = rstd if it == 1 else tmp.tile(
                        [128, S], F32, tag="qy1", bufs=1, name="qy1")
                    nc.vector.tensor_tensor(out=dst, in0=cq, in1=y,
                                            op=ALU.mult)
                    y = dst
            for jj in range(CH):
                cen = tmp.tile([128, S], F32, tag="cen", bufs=2, name="cen")
                nc.vector.scalar_tensor_tensor(
                    out=cen, in0=src[:, jj, :],
                    scalar=c_sb[:, shift_col + jj:shift_col + jj + 1],
                    in1=mean, op0=ALU.add, op1=ALU.subtract)
                emit(jj, cen, rstd)

        for _rep in range(repeats):
            with ExitStack() as s_ac:
                apool = s_ac.enter_context(tc.tile_pool(name="apool", bufs=1))
                xT = apool.tile([128, CH, S], F32R, tag="xT", bufs=1, name="xT")
                xT8 = apool.tile([128, CH, S], F8, tag="xT8", bufs=1, name="xT8")
                nc.sync.dma_start(out=xT, in_=fmaj(xT_d))
                nc.sync.dma_start(out=xT8, in_=fmaj(xT8_d))
                qT = apool.tile([128, CH, S], BF16, tag="qT", bufs=1, name="qT")
                kT = apool.tile([128, CH, S], BF16, tag="kT", bufs=1, name="kT")
                v8 = apool.tile([128, T, NH, DH], F8, tag="v8", bufs=2, name="v8")
                ctx8 = apool.tile([128, CH, S], F8, tag="ctx8", bufs=1, name="ctx8")

                wpool = s_ac.enter_context(tc.tile_pool(name="wpool", bufs=1))
                wo_sb = wpool.tile([128, CH, CH, 128], F8, name="wo_sb")

                psP = s_ac.enter_context(
                    tc.tile_pool(name="psP", bufs=1, space="PSUM"))
                wqkpool = s_ac.enter_context(tc.tile_pool(name="wqk", bufs=3))

                def project_qk(j):
                    wt = wqkpool.tile([128, 2, CH, 128], F8, tag="wqk",
                                      name="wt")
                    nc.sync.dma_start(out=wt, in_=wqk_d[j])
                    for ci, (dest, bcol) in enumerate(((qT, BQ), (kT, BK))):
                        ps = psP.tile([128, S], F32, tag="pj", bufs=1,
                                      name="ps_qk")
                        for i in range(0, CH, 2):
                            nc.tensor.matmul(
                                ps[:, :], wt[:, ci, i:i + 2, :],
                                xT8[:, i:i + 2, :],
                                start=(i == 0), stop=(i == CH - 2),
                                perf_mode=PM.DoubleRow)
                        nc.vector.tensor_scalar(
                            out=dest[:, j, :], in0=ps[:, :], scalar1=ISW,
                            scalar2=c_sb[:, bcol + j:bcol + j + 1],
                            op0=ALU.mult, op1=ALU.add)

                # ---- V projection (token-major), then Q/K of pair 0 ----
                with ExitStack() as s_v:
                    wvpool = s_v.enter_context(
                        tc.tile_pool(name="wvp", bufs=1))
                    wv_sb = wvpool.tile([128, CH, H], F8, name="wv_sb")
                    nc.sync.dma_start(out=wv_sb, in_=fmaj(wv_d))
                    psV = s_v.enter_context(
                        tc.tile_pool(name="psV", bufs=1, space="PSUM"))
                    for t in range(T):
                        for half in range(2):
                            ps = psV.tile([128, 512], F32, tag="mv", bufs=4,
                                          name="ps_v")
                            for i in range(0, CH, 2):
                                nc.tensor.matmul(
                                    ps[:, 0:384],
                                    xT8[:, i:i + 2, ts(t, 128)],
                                    wv_sb[:, i:i + 2, ts(half, 384)],
                                    start=(i == 0), stop=(i == CH - 2),
                                    perf_mode=PM.DoubleRow)
                            nc.vector.tensor_scalar(
                                out=v8[:, t, ts(half, 6), :].rearrange(
                                    "p h d -> p (h d)"),
                                in0=ps[:, 0:384],
                                scalar1=c_sb[:, MCOL + t:MCOL + t + 1],
                                scalar2=None, op0=ALU.mult)
                    project_qk(0)

                # ---- pair loop: scores -> exp -> den/ctx ----
                with ExitStack() as s_b:
                    psS = s_b.enter_context(
                        tc.tile_pool(name="psS", bufs=1, space="PSUM"))
                    psD = s_b.enter_context(
                        tc.tile_pool(name="psD", bufs=1, space="PSUM"))
                    bpool = s_b.enter_context(
                        tc.tile_pool(name="bpool", bufs=1))
                    for j in range(NP):
                        if j > 0:
                            project_qk(j)
                        if j == 2:
                            nc.sync.dma_start(
                                out=wo_sb,
                                in_=wob_d.rearrange("j p i m -> p j i m"))
                        es_ab = []
                        for half in range(2):
                            es = bpool.tile([128, T, S], F8, tag=f"es{half}",
                                            bufs=3, name=f"es{half}")
                            es_ab.append(es)
                            for u in range(2):
                                sc_ps = psS.tile([128, 1024], F32, tag="sc",
                                                 bufs=2, name="sc_ps")
                                for v_ in range(2):
                                    t = 2 * u + v_
                                    nc.tensor.matmul(
                                        sc_ps[:, ts(v_, 512)],
                                        kT[ts(half, 64), j, ts(t, 128)],
                                        qT[ts(half, 64), j, :],
                                        start=True, stop=True,
                                        tile_position=(half * 64, 0))
                                nc.scalar.activation(
                                    out=es[:, 2 * u:2 * u + 2, :].rearrange(
                                        "p a b -> p (a b)"),
                                    in_=sc_ps[:, :], func=AF.Exp, scale=0.125)
                        # den: plain-fp8 col-tiled (dst base 64 legal);
                        # ctx: DoubleRow per head into base-0 banks, then
                        # partition-shifted multiplies (PSUM in0 base may
                        # differ from SBUF in1/out base).
                        den_ps = psD.tile([128, S], F32, tag="den", bufs=1,
                                          name="den_ps")
                        cx = []
                        for half, es in enumerate(es_ab):
                            for t in range(T):
                                nc.tensor.matmul(
                                    den_ps[ts(half, 64), :],
                                    mden[:, t, :], es[:, t, :],
                                    start=(t == 0), stop=(t == T - 1),
                                    tile_position=(0, half * 64))
                            ctx_ps = psD.tile([128, S], F32, tag="ctx",
                                              bufs=2, name="ctx_ps")
                            cx.append(ctx_ps)
                            for u in range(2):
                                nc.tensor.matmul(
                                    ctx_ps[0:64, :],
                                    v8[:, 2 * u:2 * u + 2, 2 * j + half, :],
                                    es[:, 2 * u:2 * u + 2, :],
                                    start=(u == 0), stop=(u == 1),
                                    perf_mode=PM.DoubleRow)
                        recip = bpool.tile([128, S], F32, tag="recip", bufs=2,
                                           name="recip")
                        nc.vector.reciprocal_approx_fast(out=recip,
                                                         in_=den_ps[:, :])
                        for half in range(2):
                            nc.vector.tensor_tensor(
                                out=ctx8[ts(half, 64), j, :],
                                in0=cx[half][0:64, :],
                                in1=recip[ts(half, 64), :], op=ALU.mult)

                # ---- Wo + residual + LN1 ----
                r1 = mid.tile([128, CH, S], F32R, name="r1")
                q32 = mid.tile([128, CH, S], F32, name="q32")
                QDT = F8 if ffn_mode in ("hilo", "mix") else BF16
                q8 = mid.tile([128, CH, S], QDT, name="q8")
                if ffn_mode == "hilo":
                    qlo = mid.tile([128, CH, S], F8, name="qlo")
                with ExitStack() as s_c:
                    psC = s_c.enter_context(
                        tc.tile_pool(name="psC", bufs=1, space="PSUM"))
                    for jj in range(CH):
                        ps = psC.tile([128, S], F32, tag="mw", bufs=5,
                                      name="ps_wo")
                        for i in range(0, CH, 2):
                            nc.tensor.matmul(
                                ps[:, :], wo_sb[:, jj, i:i + 2, :],
                                ctx8[:, i:i + 2, :],
                                start=(i == 0), stop=(i == CH - 2),
                                perf_mode=PM.DoubleRow)
                        nc.vector.scalar_tensor_tensor(
                            out=r1[:, jj, :], in0=ps[:, :],
                            scalar=1.0 / (SW * SW), in1=xT[:, jj, :],
                            op0=ALU.mult, op1=ALU.add)

                    def emit_ln1(jj, cen, rstd):
                        nc.vector.scalar_tensor_tensor(
                            out=q32[:, jj, :], in0=cen,
                            scalar=c_sb[:, L1G + jj:L1G + jj + 1], in1=rstd,
                            op0=ALU.mult, op1=ALU.mult)
                        nc.scalar.activation(out=q8[:, jj, :],
                                             in_=q32[:, jj, :],
                                             func=AF.Identity)
                        if ffn_mode == "hilo":
                            nc.gpsimd.tensor_tensor(
                                out=qlo[:, jj, :], in0=q32[:, jj, :],
                                in1=q8[:, jj, :], op=ALU.subtract)

                    layer_norm(psC, BO, 0, r1, emit_ln1)

            # ---- FFN + LN2 ----
            with ExitStack() as s_de:
                GDT = F8 if ffn_mode == "hilo" else BF16
                gel8 = fpool.tile([128, CF, S], GDT, tag="gel8", bufs=1,
                                  name="gel8")
                psF = s_de.enter_context(
                    tc.tile_pool(name="psF", bufs=1, space="PSUM"))
                for fg in range(CF // 4):
                    w1t = w1pool.tile([128, 4, W1P, 128], W1DT, tag="w1",
                                      name="w1t")
                    nc.scalar.dma_start(
                        out=w1t,
                        in_=w1_d.rearrange("f p i m -> p f i m")[
                            :, 4 * fg:4 * fg + 4, :, :])
                    for ff in range(4):
                        f = 4 * fg + ff
                        ps = psF.tile([128, S], F32, tag="m1", bufs=3,
                                      name="ps_f1")
                        if ffn_mode == "hilo":
                            # hi@q8 + lo@q8 + hi@qlo  (drop lo@qlo)
                            for i in range(0, CH, 2):
                                nc.tensor.matmul(
                                    ps[:, :], w1t[:, ff, i:i + 2, :],
                                    q8[:, i:i + 2, :],
                                    start=(i == 0), stop=False,
                                    perf_mode=PM.DoubleRow)
                            for i in range(0, CH, 2):
                                nc.tensor.matmul(
                                    ps[:, :], w1t[:, ff, CH + i:CH + i + 2, :],
                                    q8[:, i:i + 2, :],
                                    start=False, stop=False,
                                    perf_mode=PM.DoubleRow)
                            for i in range(0, CH, 2):
                                nc.tensor.matmul(
                                    ps[:, :], w1t[:, ff, i:i + 2, :],
                                    qlo[:, i:i + 2, :],
                                    start=False, stop=(i == CH - 2),
                                    perf_mode=PM.DoubleRow)
                        elif ffn_mode == "mix":
                            for i in range(0, CH, 2):
                                nc.tensor.matmul(
                                    ps[:, :], w1t[:, ff, i:i + 2, :],
                                    q8[:, i:i + 2, :],
                                    start=(i == 0), stop=(i == CH - 2),
                                    perf_mode=PM.DoubleRow)
                        else:
                            for i in range(CH):
                                nc.tensor.matmul(
                                    ps[:, :], w1t[:, ff, i, :],
                                    q8[:, i, :],
                                    start=(i == 0), stop=(i == CH - 1))
                        nc.scalar.activation(
                            out=gel8[:, f, :], in_=ps[:, :], func=AF.Gelu,
                            scale=ISW if ffn_mode in ("hilo", "mix") else 1.0,
                            bias=c_sb[:, B1E + f:B1E + f + 1])

                r2 = mid.tile([128, CH, S], F32R, name="r2")

                def _ffn2_chunk(psF, w2t, jq, jj, r2):
                    ps = psF.tile([128, S], F32, tag="m2", bufs=3,
                                  name="ps_f2")
                    if ffn_mode == "hilo":
                        for i in range(0, CF, 2):
                            nc.tensor.matmul(
                                ps[:, :], w2t[:, jq, i:i + 2, :],
                                gel8[:, i:i + 2, :],
                                start=(i == 0), stop=False,
                                perf_mode=PM.DoubleRow)
                        for i in range(0, CF, 2):
                            nc.tensor.matmul(
                                ps[:, :], w2t[:, jq, CF + i:CF + i + 2, :],
                                gel8[:, i:i + 2, :],
                                start=False, stop=(i == CF - 2),
                                perf_mode=PM.DoubleRow)
                    else:
                        for i in range(CF):
                            nc.tensor.matmul(
                                ps[:, :], w2t[:, jq, i, :], gel8[:, i, :],
                                start=(i == 0), stop=(i == CF - 1))
                    sc1 = ISW if ffn_mode == "hilo" else 1.0
                    nc.vector.scalar_tensor_tensor(
                        out=r2[:, jj, :], in0=ps[:, :], scalar=sc1,
                        in1=q32[:, jj, :], op0=ALU.mult, op1=ALU.add)

                for jg in range(CH // 2):
                    w2t = w2pool.tile([128, 2, W2P, 128], W2DT, tag="w2",
                                      name="w2t")
                    nc.scalar.dma_start(
                        out=w2t,
                        in_=w2_d.rearrange("j p i m -> p j i m")[
                            :, 2 * jg:2 * jg + 2, :, :])
                    for jq in range(2):
                        jj = 2 * jg + jq
                        _ffn2_chunk(psF, w2t, jq, jj, r2)
                def emit_ln2(jj, cen, rstd):
                    nrm = tmp.tile([128, S], F32, tag="nrm2", bufs=2,
                                   name="nrm2")
                    nc.gpsimd.tensor_tensor(out=nrm, in0=cen, in1=rstd,
                                            op=ALU.mult)
                    ot = tmp.tile([128, S], F32, tag="ot", bufs=2, name="ot")
                    nc.scalar.activation(
                        out=ot, in_=nrm, func=AF.Identity,
                        bias=c_sb[:, L2B + jj:L2B + jj + 1],
                        scale=c_sb[:, L2G + jj:L2G + jj + 1])
                    nc.sync.dma_start(out=out_d[ts(jj, 128), :], in_=ot)

                layer_norm(psF, C2, 1, r2, emit_ln2, cen_pool=True)

    nc.finalize()
    return nc


_NC_CACHE = None


def _get_nc():
    global _NC_CACHE
    if _NC_CACHE is None:
        _NC_CACHE = build_nc()
    return _NC_CACHE


def make_in_maps(hidden_states, attention_mask, Wq, bq, Wk, bk, Wv, bv, Wo, bo,
                 ln1_g, ln1_b, W1, b1, W2, b2, ln2_g, ln2_b):
    """Host-side sharding + layout prep. Returns one input map per core."""
    f32 = np.float32
    f8 = ml_dtypes.float8_e4m3fn
    bf16np = ml_dtypes.bfloat16
    Wq, Wk, Wv, Wo = (np.asarray(w, f32) for w in (Wq, Wk, Wv, Wo))
    W1, W2 = np.asarray(W1, f32), np.asarray(W2, f32)
    bo_eff = np.asarray(bo, f32) + np.asarray(bv, f32) @ Wo
    beta1 = np.asarray(ln1_b, f32)
    b1_eff = np.asarray(b1, f32) + beta1 @ W1
    c2 = np.asarray(b2, f32) + beta1

    def blocks(w, co, ci):
        # [ci*128, co*128] -> [co, 128(k), ci, 128(m)] fp8, scaled
        return np.ascontiguousarray(
            (w * SW).reshape(ci, 128, co, 128).transpose(2, 1, 0, 3)
        ).astype(f8)

    def blocks_bf16(w, co, ci):
        return np.ascontiguousarray(
            w.reshape(ci, 128, co, 128).transpose(2, 1, 0, 3)
        ).astype(bf16np)

    def blocks_hilo(w, co, ci):
        # hi/lo residual split: [co, 128(k), 2*ci, 128(m)] fp8
        ws = (w * SW).reshape(ci, 128, co, 128).transpose(2, 1, 0, 3)
        hi = ws.astype(f8)
        lo = (ws - hi.astype(np.float32)).astype(f8)
        return np.ascontiguousarray(np.concatenate([hi, lo], axis=2))

    wqkb = np.ascontiguousarray(
        np.stack([blocks(Wq, CH, CH), blocks(Wk, CH, CH)], axis=2))
    wob = blocks(Wo, CH, CH)
    if FFN_MODE == "hilo":
        w1b = blocks_hilo(W1, CF, CH)
        w2b = blocks_hilo(W2, CH, CF)
    elif FFN_MODE == "mix":
        w1b = blocks(W1, CF, CH)
        w2b = blocks_bf16(W2, CH, CF)
    else:
        w1b = blocks_bf16(W1, CF, CH)
        w2b = blocks_bf16(W2, CH, CF)
    wv8 = (Wv * SW).astype(f8)

    def cols(v, n):
        return np.ascontiguousarray(np.asarray(v, f32).reshape(n, 128).T)

    base = np.zeros((128, NCONST), f32)
    base[:, BQ:BQ + CH] = cols(bq, CH)
    base[:, BK:BK + CH] = cols(bk, CH)
    base[:, BO:BO + CH] = cols(bo_eff, CH)
    base[:, L1G:L1G + CH] = cols(ln1_g, CH)
    base[:, C2:C2 + CH] = cols(c2, CH)
    base[:, L2G:L2G + CH] = cols(ln2_g, CH)
    base[:, L2B:L2B + CH] = cols(ln2_b, CH)
    base[:, B1E:B1E + CF] = cols(b1_eff, CF)

    scal = np.zeros((128, 2), f32)
    scal[:, 0] = bo_eff.sum() / H
    scal[:, 1] = c2.sum() / H

    mask = np.asarray(attention_mask, f32)  # [B, S]
    x = np.asarray(hidden_states, f32)
    in_maps = []
    for b in range(B):
        consts = base.copy()
        consts[:, MCOL:MCOL + T] = cols(mask[b] * ISW, T)
        mden = np.ascontiguousarray(
            np.broadcast_to(
                (mask[b] * ISW).reshape(T, 128, 1), (T, 128, 64)
            ).transpose(1, 0, 2)
        ).astype(f8)
        xT = np.ascontiguousarray(x[b].T)
        in_maps.append({
            "xT": xT, "xT8": xT.astype(f8),
            "wqkb": wqkb, "wv": wv8, "wob": wob,
            "w1b": w1b, "w2b": w2b,
            "mden": mden, "consts": consts, "scal": scal,
        })
    return in_maps


def kernel(**inputs):
    nc = _get_nc()
    in_maps = make_in_maps(**inputs)
    res = run_bass_kernel_spmd(nc, in_maps, core_ids=list(range(B)))
    out = np.stack([np.ascontiguousarray(r["outT"].T) for r in res.results])
    return out.astype(np.float32)

